# revision 7
# baseline (speedup 1.0000x reference)
"""DFlashAttention Trainium2 kernel (8-core tensor-parallel over attention heads).

Shapes (hardcoded): D=2048, N=16 q-heads, K=8 kv-heads, H=128,
T_NOISE=2048 (query tokens), T_CTX=4096, S=6144 (kv tokens).

Sharding: core c owns q-heads {2c, 2c+1} and kv-head c (GQA groups=2).
Each core computes a partial (T, D) output (its 2 heads' slice of the
o-projection contraction); the host sums the 8 partials (TP unshard).

Layout strategy per core:
  - x_all^T [D, S] fed replicated (d on partitions = matmul contraction dim).
  - kv proj:  psum[s,0:128]=k, psum[s,128:256]=v  (one fp32r matmul chain,
    moving free dim 256).
  - RMSNorm over H via ACT Square+accum_out; RoPE via on-device sin/cos
    (angle mod 2pi + range wrap + ACT Sin); tables built once for all 48
    token tiles.
  - attention in [s, t] orientation: scores^T = kT.T @ qT (contraction H=128,
    single matmul per (s-tile, t-chunk)); exp on ACT (scale=1/sqrt(H) folded);
    no max subtraction (|score| <= sqrt(H)*1.1^2 ~ 13.7 after RMSNorm, exp is
    safe in fp32); row-sums via ones-matmul; A@V accumulates over s-tiles in
    PSUM with V in natural [s, h] layout.
  - softmax division deferred past the o-projection (denominator is constant
    along the contraction), where it is a per-partition scalar multiply.
"""

import sys

for _p in ("/opt/trn_rl_repo", "/root/.axon_site/_ro/trn_rl_repo"):
    if _p not in sys.path:
        sys.path.append(_p)

import math
import numpy as np

import concourse.bass as bass
import concourse.tile as tile
from concourse import bacc
from concourse import mybir
from concourse.bass_utils import run_bass_kernel_spmd
from concourse.masks import make_identity

D = 2048
N_HEADS = 16
K_HEADS = 8
H = 128
T_NOISE = 2048
T_CTX = 4096
S_ALL = T_CTX + T_NOISE          # 6144
EPS = 1e-6
ROPE_THETA = 1e6
N_CORES = 8
HEADS_PER_CORE = N_HEADS // N_CORES   # 2

P = 128                       # partition dim
S_TILES = S_ALL // P          # 48
T_TILES = T_NOISE // P        # 16
NOISE_TILE0 = T_CTX // P      # 32  (noise tokens are s-tiles 32..47)
D_TILES = D // P              # 16
FREE = 512                    # moving free-dim chunk
T_CHUNKS = T_NOISE // FREE    # 4
S_CHUNKS = S_ALL // FREE      # 12
D_CHUNKS = D // FREE          # 4

F32 = mybir.dt.float32
F32R = mybir.dt.float32r
MM_DT = F32R                  # dtype for all matmul operands

TWO_PI = 2.0 * math.pi
INV_SQRT_H = 1.0 / math.sqrt(H)

_CACHE = {}


def _build_program(reps=1):
    """Build the single-core SPMD bass program. Returns (nc, out_name).
    reps>1 repeats the whole kernel body (timing harness only)."""
    nc = bacc.Bacc("TRN2", target_bir_lowering=False, debug=False,
                   num_devices=N_CORES)

    xT = nc.dram_tensor("xT", [D, S_ALL], MM_DT, kind="ExternalInput").ap()
    wkv = nc.dram_tensor("wkv", [D, 2 * H], MM_DT, kind="ExternalInput").ap()
    wq = nc.dram_tensor("wq", [D, HEADS_PER_CORE * H], MM_DT,
                        kind="ExternalInput").ap()
    wo = nc.dram_tensor("wo", [HEADS_PER_CORE, H, D], MM_DT,
                        kind="ExternalInput").ap()
    posr = nc.dram_tensor("posr", [S_TILES, P, 1], F32,
                          kind="ExternalInput").ap()
    invfb = nc.dram_tensor("invfb", [P, H // 2], F32,
                           kind="ExternalInput").ap()
    qscaleb = nc.dram_tensor("qscaleb", [P, H], F32,
                             kind="ExternalInput").ap()
    kscaleb = nc.dram_tensor("kscaleb", [P, H], F32,
                             kind="ExternalInput").ap()
    onesb = nc.dram_tensor("onesb", [P, 1], MM_DT, kind="ExternalInput").ap()
    out = nc.dram_tensor("out", [T_NOISE, D], F32, kind="ExternalOutput").ap()

    with tile.TileContext(nc) as tc:
        for rep in range(reps):
            _emit(nc, tc, xT, wkv, wq, wo, posr, invfb, qscaleb, kscaleb,
                  onesb, out, pfx=f"r{rep}_")
    nc.compile()
    return nc, "out"


def _emit(nc, tc, xT, wkv, wq, wo, posr, invfb, qscaleb, kscaleb, onesb, out, pfx=""):
    import contextlib
    ctx = contextlib.ExitStack()
    with ctx:
        ctx.enter_context(nc.named_scope(pfx + "setup"))
        const = ctx.enter_context(tc.tile_pool(name=pfx + "const", bufs=1))
        persist = ctx.enter_context(tc.tile_pool(name=pfx + "persist", bufs=1))

        # ---- constants ----
        ident = const.tile([P, P], F32, tag="ident")
        make_identity(nc, ident[:])
        ones = const.tile([P, 1], MM_DT, tag="ones")
        nc.sync.dma_start(ones[:], onesb[:])
        invf_sb = const.tile([P, H // 2], F32, tag="invf")
        nc.sync.dma_start(invf_sb[:], invfb[:])
        qsc_sb = const.tile([P, H], F32, tag="qsc")
        nc.sync.dma_start(qsc_sb[:], qscaleb[:])
        ksc_sb = const.tile([P, H], F32, tag="ksc")
        nc.sync.dma_start(ksc_sb[:], kscaleb[:])
        eps_col = const.tile([P, 1], F32, tag="eps")
        nc.vector.memset(eps_col[:], EPS)
        pos_sb = const.tile([P, S_TILES], F32, tag="pos")
        for si in range(S_TILES):
            nc.sync.dma_start(pos_sb[:, si:si + 1], posr[si])

        wkv_sb = [const.tile([P, 2 * H], MM_DT, tag=f"wkv{d}", name=f"wkv{d}")
                  for d in range(D_TILES)]
        wq_sb = [const.tile([P, HEADS_PER_CORE * H], MM_DT, tag=f"wq{d}", name=f"wqs{d}")
                 for d in range(D_TILES)]
        for d in range(D_TILES):
            nc.sync.dma_start(wkv_sb[d][:], wkv[d * P:(d + 1) * P, :])
            nc.sync.dma_start(wq_sb[d][:], wq[d * P:(d + 1) * P, :])
        wo_sb = [const.tile([P, D], MM_DT, tag=f"wo{h}", name=f"wos{h}")
                 for h in range(HEADS_PER_CORE)]
        for h in range(HEADS_PER_CORE):
            nc.sync.dma_start(wo_sb[h][:], wo[h])

        # ---- persistent activations ----
        half = H // 2
        sin_all = persist.tile([P, S_TILES * half], F32, tag="sin")
        cos_all = persist.tile([P, S_TILES * half], F32, tag="cos")
        kT_sb = persist.tile([P, S_ALL], MM_DT, tag="kT")
        v_sb = persist.tile([P, S_ALL], MM_DT, tag="v")       # [s-tile, h] blocks
        qT_sb = persist.tile([P, HEADS_PER_CORE * T_NOISE], MM_DT, tag="qT")
        oT_sb = persist.tile([P, HEADS_PER_CORE * T_NOISE], MM_DT, tag="oT")
        r_all = persist.tile([1, HEADS_PER_CORE * T_NOISE], F32, tag="r")
        rcol = persist.tile([P, HEADS_PER_CORE * T_TILES], F32, tag="rcol")

        # ---- RoPE sin/cos tables for all 48 token tiles ----
        # angle = pos * inv_freq; range-reduce mod 2pi via Cody-Waite
        # (k = int(angle/2pi); red = ((ang - k*c1) - k*c2) - k*c3).
        CW1, CW2, CW3 = 6.28125, 0.0019353071693331003, 1.0253131677018246e-11
        HGRP = S_TILES // 2
        HW_ = HGRP * half
        with nc.named_scope(pfx + "rope"), \
             tc.tile_pool(name=pfx + "ropebuild", bufs=1) as rp:
            for g in range(2):
                ang = rp.tile([P, HW_], F32, tag="ang", name="ang")
                kq = rp.tile([P, HW_], F32, tag="kq", name="kq")
                ki = rp.tile([P, HW_], mybir.dt.int32, tag="ki", name="ki")
                wrap = rp.tile([P, HW_], F32, tag="wrap", name="wrap")
                for j in range(HGRP):
                    si = g * HGRP + j
                    nc.vector.tensor_scalar_mul(
                        ang[:, j * half:(j + 1) * half], invf_sb[:, :],
                        pos_sb[:, si:si + 1])
                nc.vector.tensor_scalar_mul(kq[:], ang[:], 1.0 / TWO_PI)
                nc.vector.tensor_copy(ki[:], kq[:])
                nc.vector.tensor_copy(kq[:], ki[:])
                nc.vector.cody_waite_cascade(ang[:], ang[:], kq[:],
                                             CW1, CW2, CW3)
                dst = slice(g * HW_, (g + 1) * HW_)
                nc.vector.add_range_wrap(wrap[:], ang[:], 0.0, math.pi, TWO_PI)
                nc.scalar.activation(sin_all[:, dst], wrap[:],
                                     mybir.ActivationFunctionType.Sin)
                nc.vector.add_range_wrap(wrap[:], ang[:], math.pi / 2, math.pi,
                                         TWO_PI)
                nc.scalar.activation(cos_all[:, dst], wrap[:],
                                     mybir.ActivationFunctionType.Sin)

        def norm_rope_transpose(src_psum, scale_sb, si, dst_sb, work, psum_t):
            """src_psum [P(tok),H] fp32 -> rms-norm*scale -> rope -> transpose
            -> dst_sb [P(h), 128 tok]. si = token-tile index for positions."""
            sq = work.tile([P, H], F32, tag="sq")
            ssq = work.tile([P, 1], F32, tag="ssq")
            nc.scalar.activation(sq[:], src_psum, mybir.ActivationFunctionType.Square,
                                 accum_out=ssq[:])
            rms = work.tile([P, 1], F32, tag="rms")
            nc.scalar.activation(rms[:], ssq[:], mybir.ActivationFunctionType.Sqrt,
                                 bias=eps_col[:], scale=1.0 / H)
            rinv = work.tile([P, 1], F32, tag="rinv")
            nc.vector.reciprocal(rinv[:], rms[:])
            xn = work.tile([P, H], F32, tag="xn")
            nc.vector.scalar_tensor_tensor(
                xn[:], src_psum, rinv[:], scale_sb[:],
                mybir.AluOpType.mult, mybir.AluOpType.mult)
            # rope
            co = cos_all[:, si * half:(si + 1) * half]
            sn = sin_all[:, si * half:(si + 1) * half]
            x1 = xn[:, 0:half]
            x2 = xn[:, half:H]
            t1 = work.tile([P, half], F32, tag="t1")
            t2 = work.tile([P, half], F32, tag="t2")
            xr = work.tile([P, H], F32, tag="xr")
            nc.vector.tensor_mul(t1[:], x1, co)
            nc.vector.tensor_mul(t2[:], x2, sn)
            nc.vector.tensor_sub(xr[:, 0:half], t1[:], t2[:])
            nc.vector.tensor_mul(t1[:], x2, co)
            nc.vector.tensor_mul(t2[:], x1, sn)
            nc.vector.tensor_add(xr[:, half:H], t1[:], t2[:])
            # transpose -> dst
            pt = psum_t.tile([P, P], F32, tag="pt")
            nc.tensor.transpose(pt[:], xr[:], ident[:])
            nc.vector.tensor_copy(dst_sb, pt[:])

        # ---- Phase A: K/V projection, norm+rope K, build kT and v ----
        with nc.named_scope(pfx + "phaseA"), \
             tc.tile_pool(name=pfx + "pa_x", bufs=3) as xp, \
             tc.tile_pool(name=pfx + "pa_ps", bufs=1, space="PSUM") as pskv, \
             tc.tile_pool(name=pfx + "pa_pt", bufs=2, space="PSUM") as pst, \
             tc.tile_pool(name=pfx + "pa_w", bufs=2) as work:
            for sc in range(S_CHUNKS):
                xt = [None] * D_TILES
                ps = [pskv.tile([P, 2 * H], F32, tag=f"kv{j}", name=f"pskv{j}") for j in range(4)]
                for d in range(D_TILES):
                    xt[d] = xp.tile([P, FREE], MM_DT, tag="xstage", name="xstage")
                    nc.sync.dma_start(
                        xt[d][:], xT[d * P:(d + 1) * P,
                                     sc * FREE:(sc + 1) * FREE])
                    for j in range(4):
                        nc.tensor.matmul(
                            ps[j][:], xt[d][:, j * P:(j + 1) * P],
                            wkv_sb[d][:], start=(d == 0), stop=(d == D_TILES - 1))
                for j in range(4):
                    si = sc * 4 + j
                    nc.vector.tensor_copy(v_sb[:, si * P:(si + 1) * P],
                                          ps[j][:, H:2 * H])
                    norm_rope_transpose(ps[j][:, 0:H], ksc_sb, si,
                                        kT_sb[:, si * P:(si + 1) * P],
                                        work, pst)

        # ---- Phase B: Q projection, norm+rope, build qT (2 heads) ----
        with nc.named_scope(pfx + "phaseB"), \
             tc.tile_pool(name=pfx + "pb_x", bufs=3) as xp, \
             tc.tile_pool(name=pfx + "pb_ps", bufs=1, space="PSUM") as psq, \
             tc.tile_pool(name=pfx + "pb_pt", bufs=2, space="PSUM") as pst, \
             tc.tile_pool(name=pfx + "pb_w", bufs=2) as work:
            for tch in range(T_CHUNKS):
                xt = [None] * D_TILES
                ps = [psq.tile([P, HEADS_PER_CORE * H], F32, tag=f"q{j}", name=f"psq{j}")
                      for j in range(4)]
                for d in range(D_TILES):
                    xt[d] = xp.tile([P, FREE], MM_DT, tag="xstage", name="xstage")
                    nc.sync.dma_start(
                        xt[d][:], xT[d * P:(d + 1) * P,
                                     T_CTX + tch * FREE:T_CTX + (tch + 1) * FREE])
                    for j in range(4):
                        nc.tensor.matmul(
                            ps[j][:], xt[d][:, j * P:(j + 1) * P],
                            wq_sb[d][:], start=(d == 0), stop=(d == D_TILES - 1))
                for j in range(4):
                    ti = tch * 4 + j
                    for hh in range(HEADS_PER_CORE):
                        norm_rope_transpose(
                            ps[j][:, hh * H:(hh + 1) * H], qsc_sb,
                            NOISE_TILE0 + ti,
                            qT_sb[:, hh * T_NOISE + ti * P:
                                  hh * T_NOISE + (ti + 1) * P],
                            work, pst)

        # ---- Phase C: attention ----
        PAIR = 2 * FREE   # exp processes two score banks at once
        with nc.named_scope(pfx + "phaseC"), \
             tc.tile_pool(name=pfx + "pc_sc", bufs=2, space="PSUM") as psc, \
             tc.tile_pool(name=pfx + "pc_av", bufs=2, space="PSUM") as pav, \
             tc.tile_pool(name=pfx + "pc_r", bufs=2, space="PSUM") as pr, \
             tc.tile_pool(name=pfx + "pc_exp", bufs=3) as pexp:
            for hh in range(HEADS_PER_CORE):
                for tch in range(T_CHUNKS):
                    qslice = qT_sb[:, hh * T_NOISE + tch * FREE:
                                   hh * T_NOISE + (tch + 1) * FREE]
                    av = pav.tile([P, FREE], F32, tag="av")
                    rr = pr.tile([1, FREE], F32, tag="rr")
                    for sp in range(S_TILES // 2):
                        sc_ps = psc.tile([P, PAIR], F32, tag="sc")
                        ex = pexp.tile([P, PAIR], MM_DT, tag="ex")
                        for u in range(2):
                            si = sp * 2 + u
                            nc.tensor.matmul(
                                sc_ps[:, u * FREE:(u + 1) * FREE],
                                kT_sb[:, si * P:(si + 1) * P], qslice,
                                start=True, stop=True)
                        nc.scalar.activation(ex[:], sc_ps[:],
                                             mybir.ActivationFunctionType.Exp,
                                             scale=INV_SQRT_H)
                        for u in range(2):
                            si = sp * 2 + u
                            first = si == 0
                            last = si == S_TILES - 1
                            nc.tensor.matmul(
                                av[:], v_sb[:, si * P:(si + 1) * P],
                                ex[:, u * FREE:(u + 1) * FREE],
                                start=first, stop=last)
                            nc.tensor.matmul(
                                rr[:], ones[:],
                                ex[:, u * FREE:(u + 1) * FREE],
                                start=first, stop=last)
                    nc.vector.tensor_copy(
                        oT_sb[:, hh * T_NOISE + tch * FREE:
                              hh * T_NOISE + (tch + 1) * FREE], av[:])
                    nc.vector.reciprocal(
                        r_all[0:1, hh * T_NOISE + tch * FREE:
                              hh * T_NOISE + (tch + 1) * FREE], rr[:])

        # recip row -> per-partition columns (SBUF->SBUF DMA transpose, tiny)
        ctx.enter_context(nc.named_scope(pfx + "phaseD"))
        for hh in range(HEADS_PER_CORE):
            for ti in range(T_TILES):
                nc.sync.dma_start(
                    rcol[:, hh * T_TILES + ti:hh * T_TILES + ti + 1],
                    r_all[0:1, hh * T_NOISE + ti * P:hh * T_NOISE + (ti + 1) * P])

        # ---- Phase D: o-projection + deferred softmax normalization ----
        with tc.tile_pool(name=pfx + "pd_ps", bufs=2, space="PSUM") as pso, \
             tc.tile_pool(name=pfx + "pd_w", bufs=3) as work:
            for ti in range(T_TILES):
                for dc in range(D_CHUNKS):
                    po = [pso.tile([P, FREE], F32, tag=f"po{h}", name=f"po{h}")
                          for h in range(HEADS_PER_CORE)]
                    for h in range(HEADS_PER_CORE):
                        nc.tensor.matmul(
                            po[h][:],
                            oT_sb[:, h * T_NOISE + ti * P:h * T_NOISE + (ti + 1) * P],
                            wo_sb[h][:, dc * FREE:(dc + 1) * FREE],
                            start=True, stop=True)
                    tmp = work.tile([P, FREE], F32, tag="tmp")
                    nc.vector.tensor_scalar_mul(
                        tmp[:], po[1][:],
                        rcol[:, T_TILES + ti:T_TILES + ti + 1])
                    ot = work.tile([P, FREE], F32, tag="ot")
                    nc.vector.scalar_tensor_tensor(
                        ot[:], po[0][:], rcol[:, ti:ti + 1], tmp[:],
                        mybir.AluOpType.mult, mybir.AluOpType.add)
                    nc.sync.dma_start(
                        out[ti * P:(ti + 1) * P, dc * FREE:(dc + 1) * FREE],
                        ot[:])


def _get_program(reps=1):
    key = f"prog{reps}"
    if key not in _CACHE:
        _CACHE[key] = _build_program(reps)
    return _CACHE[key]


def prepare_in_maps(x_noise, target_hidden, Wq, Wk, Wv, Wo, q_scale, k_scale,
                    noise_positions, ctx_positions):
    x_noise = np.asarray(x_noise, dtype=np.float32)
    target_hidden = np.asarray(target_hidden, dtype=np.float32)
    Wq = np.asarray(Wq, dtype=np.float32)
    Wk = np.asarray(Wk, dtype=np.float32)
    Wv = np.asarray(Wv, dtype=np.float32)
    Wo = np.asarray(Wo, dtype=np.float32)
    q_scale = np.asarray(q_scale, dtype=np.float32)
    k_scale = np.asarray(k_scale, dtype=np.float32)

    x_all = np.concatenate([target_hidden, x_noise], axis=0)       # (S, D)
    xT = np.ascontiguousarray(x_all.T)                             # (D, S)
    pos_all = np.concatenate(
        [np.asarray(ctx_positions), np.asarray(noise_positions)]
    ).astype(np.float32)
    posr = np.ascontiguousarray(pos_all.reshape(S_TILES, P, 1))
    half = H // 2
    inv_freq = (ROPE_THETA ** (-np.arange(half, dtype=np.float32) * 2.0 / H)
                ).astype(np.float32)
    invfb = np.ascontiguousarray(np.broadcast_to(inv_freq, (P, half)))
    qscaleb = np.ascontiguousarray(np.broadcast_to(q_scale, (P, H)))
    kscaleb = np.ascontiguousarray(np.broadcast_to(k_scale, (P, H)))

    in_maps = []
    for c in range(N_CORES):
        wkv = np.ascontiguousarray(
            np.concatenate([Wk[:, c, :], Wv[:, c, :]], axis=1))     # (D, 256)
        wq = np.ascontiguousarray(
            Wq[:, c * HEADS_PER_CORE:(c + 1) * HEADS_PER_CORE, :]
            .reshape(D, HEADS_PER_CORE * H))                        # (D, 256)
        wo = np.ascontiguousarray(
            Wo[c * HEADS_PER_CORE:(c + 1) * HEADS_PER_CORE])        # (2,128,D)
        in_maps.append({
            "xT": xT, "wkv": wkv, "wq": wq, "wo": wo,
            "posr": posr, "invfb": invfb,
            "qscaleb": qscaleb, "kscaleb": kscaleb,
            "onesb": np.ones((P, 1), dtype=np.float32),
        })
    return in_maps


def kernel(**inputs):
    in_maps = prepare_in_maps(**inputs)
    nc, out_name = _get_program()
    res = run_bass_kernel_spmd(nc, in_maps, core_ids=list(range(N_CORES)))
    acc = np.zeros((T_NOISE, D), dtype=np.float32)
    for r in res.results:
        acc += r[out_name]
    return acc


def run_traced(inputs, **kw):
    """Run once with NTFF tracing; returns BassKernelResults (exec_time_ns)."""
    in_maps = prepare_in_maps(**inputs)
    nc, out_name = _get_program()
    return run_bass_kernel_spmd(nc, in_maps, core_ids=list(range(N_CORES)),
                                trace=True, **kw)

